# revision 105
# baseline (speedup 1.0000x reference)
"""Trainium2 Bass kernel for nn_MultiHeadAttention_45019847196962.

Reference computation (per batch b):
    q = Q @ Wq + bq                 # (Lq, H*D)
    v = V @ Wv + bv                 # (Lk, H*D)   (used as both keys and values)
    scores = q_h @ v_h^T            # per head, no 1/sqrt(d) scale
    align  = softmax(scores, -1)
    attn   = align @ v_h            # concat heads -> (Lq, H*D)
    out    = tanh([attn | Q] @ Wf + bf)

Sharding: data-parallel over batch. 16 batches / 8 cores = 2 batches per
core; weights replicated. No collectives.

Key algebraic restructuring vs the obvious dataflow:
  - bv is dropped on-device entirely: softmax rows are shift-invariant so
    dropping bv from the keys changes nothing, and align rows sum to one
    so align@(v+bv) = align@v + bv, which folds into the fc bias as
    bf' = bf + bv @ Wf[:H*D].  This removes all value-bias DVE work.
  - bq and bf' are added via K=1 rank-1 accumulation matmuls into the
    same PSUM group as the projection / fc, so no separate bias pass.
  - The softmax denominator S rides as a 65th "ones" column on the attn
    matmul stationary (align rows summing to 1 make this exact).
  - attn normalization (x 1/S) happens on the DVE as a tensor-tensor
    multiply against a PE-broadcast r row, fused with the PSUM drain.

Input staging (v2): Q/V/weights are loaded as PLAIN fp32 DMAs spread
across all three DMA rings (sync + scalar HWDGE, gpsimd SWDGE), then
Q/V are transposed ON the PE via identity matmuls (fp32, 2 cyc/row)
into PSUM, and the PSUM drain doubles as the fp32->bf16 cast on the
DVE.  This replaces the previous serial gpsimd cast-DMA queue + XBAR
DMA-transpose pipeline which kept the PE starved for the first ~60us.
Batch 0's pair-0 projection trio is emitted inline right after its
transposes so the first exp fires ~11us in; everything else (rest of
b0's projections, b1's transposes + projections, b0's fc) flows
through an ordered filler queue popped between exp-gated attention
slots, with ensure() barriers guaranteeing emission order (Tile deps
follow program order, so a consumer must be emitted after its
producer units).

HW facts this kernel leans on (verified by trace):
  - Plain HWDGE/SWDGE DMAs run ~250-320GB/s per ring; dtype-casting
    DMAs and XBAR transposes are far slower / serialize on one ring.
  - PE transpose matmuls cost ~2 cycles/row fp32 (~107ns per 128x128
    tile warm), nearly free in the startup window where the PE idles.
  - exp on the Scalar engine is ~1.15us per [128,2,512] slab; 64 slabs
    = ~73us/core, slightly under the PE's total work, so the kernel is
    PE-bound and the scheduling goal is a dense PE stream.
  - The PE HAM clock-gate needs ~3.4us of activity to reach full
    clock: a short dummy-matmul warmup covers the initial DMA phase.
  - Tiny multi-partition DMAs (<=32B/partition) take ~14us to signal
    completion; bv's load is issued early but its cast is emitted at
    the batch hinge, and the 1/S gather uses a [16,64] shape.
"""

import numpy as np

B, LQ, LK = 16, 512, 1024
F, H, D = 512, 8, 64
NCORES = 8
BPC = B // NCORES  # batches per core

_CACHE = {}


def _split_sync_waits(nc, mybir, maxw=1):
    """This container's walrus rejects instructions with more than one sync
    wait ("Too many sync wait commands").  Move excess waits onto NoOp
    instructions inserted just before the over-subscribed instruction on the
    same engine queue (program order preserves the wait semantics)."""
    for fn in nc.m.functions:
        for blk in fn.blocks:
            insts = blk.instructions
            i = 0
            while i < len(insts):
                inst = insts[i]
                si = getattr(inst, "sync_info", None)
                if si is not None and len(si.on_wait) > maxw:
                    waits = list(si.on_wait)
                    del si.on_wait[maxw:]
                    pre = []
                    for j in range(maxw, len(waits), maxw):
                        nop = mybir.InstNoOp(
                            name=nc.get_next_instruction_name(),
                            engine=inst.engine,
                            ins=[],
                            outs=[],
                            sync_info=mybir.SyncInfo(
                                on_wait=waits[j:j + maxw], on_update=[]),
                        )
                        pre.append(nop)
                    insts[i:i] = pre
                    i += len(pre)
                i += 1


def _patch_sem_clear_chunking(bass, chunk=16):
    """walrus here rejects the kernel-tail SEM_RANGE_CLEAR ISA op when the
    semaphore range is large ("ISA wrong length").  Chunk the ranges."""
    if getattr(bass.Bass.clear_and_free_semaphores, "_chunked", False):
        return
    orig = bass.Bass.clear_and_free_semaphores

    def chunked(self, sems):
        sems = list(sems)
        nums = [s.num if hasattr(s, "num") else s for s in sems]
        order = sorted(range(len(sems)), key=lambda i: nums[i])
        for j in range(0, len(sems), chunk):
            orig(self, [sems[i] for i in order[j:j + chunk]])

    chunked._chunked = True
    bass.Bass.clear_and_free_semaphores = chunked


class _FillQ:
    """Ordered filler-unit queue.  Units are (name, closure) emitted in
    FIFO order; ensure() pops until the named unit has been emitted, which
    makes consumer emission safe regardless of pop tuning (Tile deps follow
    program order)."""

    def __init__(self):
        self.q = []
        self.done = set()

    def push(self, name, fn):
        self.q.append((name, fn))

    def pop(self, k=1):
        for _ in range(k):
            if not self.q:
                return
            name, fn = self.q.pop(0)
            fn()
            self.done.add(name)

    def ensure(self, *names):
        while self.q and any(n not in self.done for n in names):
            self.pop(1)

    def drain(self):
        self.pop(len(self.q))


def _build():
    import concourse.bass as bass
    import concourse.tile as tile
    from concourse import mybir
    from concourse import masks

    _patch_sem_clear_chunking(bass)

    dt = mybir.dt
    f32, bf16 = dt.float32, dt.bfloat16
    AF = mybir.ActivationFunctionType
    OP = mybir.AluOpType

    nc = bass.Bass("TRN2", target_bir_lowering=False, debug=False,
                   num_devices=NCORES)

    Qd = nc.dram_tensor("Q", [BPC, LQ, F], f32, kind="ExternalInput").ap()
    Vd = nc.dram_tensor("V", [BPC, LK, F], f32, kind="ExternalInput").ap()
    Wqd = nc.dram_tensor("Wq", [F, H * D], f32, kind="ExternalInput").ap()
    bqd = nc.dram_tensor("bq", [H * D], f32, kind="ExternalInput").ap()
    Wvd = nc.dram_tensor("Wv", [F, H * D], f32, kind="ExternalInput").ap()
    bvd = nc.dram_tensor("bv", [H * D], f32, kind="ExternalInput").ap()
    Wfd = nc.dram_tensor("Wf", [F + H * D, F], f32, kind="ExternalInput").ap()
    bfd = nc.dram_tensor("bf", [F], f32, kind="ExternalInput").ap()
    Od = nc.dram_tensor("O", [BPC, LQ, F], f32, kind="ExternalOutput").ap()

    with tile.TileContext(nc) as tc:
        import contextlib
        with contextlib.ExitStack() as ctx:
            def pool(name, bufs, space="SBUF"):
                return ctx.enter_context(
                    tc.tile_pool(name=name, bufs=bufs, space=space))

            const_p = pool("const", 1)
            qt_p = pool("qt", 2)        # Q^T bf16 (from PE transpose)
            vt_p = pool("vt", 4)        # V^T, one tile per (batch, lk-half)
            qproj_p = pool("qproj", 2)  # qT
            vproj_p = pool("vproj", 2)  # vT
            vn_p = pool("vn", 2)        # v natural (+ones col)
            e_p = pool("E", 2)          # exp(scores^T) per pair
            at_p = pool("attnT", 2)
            s_p = pool("s_sb", 2)
            au_p = pool("au", 4)
            s4_p = pool("s4", 3)
            r4_p = pool("r4", 3)
            r0_p = pool("r0", 3)
            rbc_p = pool("rbc", 2)
            ao_p = pool("anodd", 2)
            osb_p = pool("osb", 2)

            # fp32 input staging (plain DMAs, cast happens in PSUM drain)
            vstg_p = pool("vstg", 2)    # V halves [128,4,512] f32
            qstg_p = pool("qstg", 1)    # Q batches [128,4,512] f32
            wst_p = pool("wstage", 2)   # Wq/Wv then Wf halves, f32

            # PSUM: 8 banks of [128, 512] f32.
            ps_sc = pool("ps_sc", 2, space="PSUM")  # scores [128,2,512]: 4
            ps_at = pool("ps_at", 2, space="PSUM")  # attn out [128,512]: 2
            ps_f = pool("ps_f", 2, space="PSUM")    # proj/fc/transpose: 2

            # ---- DMA issues, ordered by need per ring ----
            # identity for PE transposes FIRST on the gpsimd engine (no
            # deps, so Tile can't push it behind a blocked DMA issue)
            ident = const_p.tile([128, 128], f32)
            masks.make_identity(nc, ident[:])
            # Wq rides the gpsimd ring ahead of the batch-1 cast-DMAs:
            # it lands ~17us (vs ~21 as the 4th sync-ring load) and the
            # cast-DMAs then queue behind it on the same FIFO.
            Wq_f32 = wst_p.tile([128, 4, H * D], f32, name="wstage",
                                tag="wstage")
            nc.gpsimd.dma_start(
                Wq_f32[:], Wqd.rearrange("(ko p) n -> p ko n", p=128))

            vstg = {}
            qstg = [None]

            def v_load(engine, b, h):
                t = vstg_p.tile([128, 4, 512], f32, name="vstg", tag="vstg")
                engine.dma_start(
                    t[:],
                    Vd[b][h * 512:(h + 1) * 512].rearrange(
                        "(ro p) f -> p ro f", p=128))
                vstg[(b, h)] = t

            # Batch 0 inputs + Wq/Wv: plain fp32 loads on the two HWDGE
            # rings (V0/Q0 then PE-transposed; weights DVE-cast).
            v_load(nc.sync, 0, 0)     # sync ring
            v_load(nc.scalar, 0, 1)   # scalar ring
            qstg[0] = qstg_p.tile([128, 4, 512], f32, name="qstg",
                                  tag="qstg")
            nc.sync.dma_start(
                qstg[0][:], Qd[0].rearrange("(ro p) f -> p ro f", p=128))
            Wv_f32 = wst_p.tile([128, 4, H * D], f32, name="wstage",
                                tag="wstage")
            nc.scalar.dma_start(
                Wv_f32[:], Wvd.rearrange("(ko p) n -> p ko n", p=128))

            # bq in qT layout [128,4] (per-partition bias column) via a
            # tiny gpsimd DMA -- the ~14us completion-signal latency is
            # fine since it's needed only at the first q drain (~28us).
            bq_f32 = const_p.tile([128, 4], f32)
            nc.gpsimd.dma_start(
                bq_f32[:], bqd.rearrange("(ko p) -> p ko", p=128))
            # bf/bv: tiny multi-partition DMAs with multi-us descriptor-gen
            # cost -- keep them off the busy HWDGE rings (gpsimd instead).
            bf_row = const_p.tile([1, F], f32)
            nc.gpsimd.dma_start(bf_row[:],
                                bfd.rearrange("(a n) -> a n", a=1))
            bv_f32 = const_p.tile([128, 4], f32)
            nc.gpsimd.dma_start(
                bv_f32[:], bvd.rearrange("(ko p) -> p ko", p=128))

            # Batch 1 inputs + Wf: fp32->bf16 cast-DMAs on the async SWDGE
            # queue (none needed before ~45us; zero PE/DVE cost).  Their
            # transfers hog the DMA fabric and stall the HWDGE rings, so a
            # tiny gpsimd op that depends on the LAST critical plain load
            # (Wq) fences their issue until the startup loads are done.
            gp_scr = const_p.tile([1, 64], f32)
            nc.gpsimd.tensor_copy(gp_scr[:], Wq_f32[0:1, 0, 0:64])
            Vbf1 = nc.dram_tensor("Vbf1", [2, LK // 2, F], bf16).ap()
            Qbf1 = nc.dram_tensor("Qbf1", [LQ, F], bf16).ap()
            nc.gpsimd.dma_start(Vbf1[0], Vd[1][0:512])
            nc.gpsimd.dma_start(Vbf1[1], Vd[1][512:1024])
            nc.gpsimd.dma_start(Qbf1[:], Qd[1])
            Wf_sb = const_p.tile([128, 8, F], bf16)
            for wh in range(2):
                nc.gpsimd.dma_start(
                    Wf_sb[:, wh * 4:(wh + 1) * 4, :],
                    Wfd[wh * 512:(wh + 1) * 512].rearrange(
                        "(ko p) n -> p ko n", p=128))

            # preload the activation table set (exp+tanh) while the input
            # DMAs are in flight so the first real exp doesn't pay ~2.2us
            act_scr = const_p.tile([1, 64], f32)

            # ---- DVE constants / casts (ordered by data arrival) ----
            ones_sb = const_p.tile([1, 64], bf16)
            nc.vector.memset(ones_sb[:], 1.0)
            ones_row = const_p.tile([1, F], bf16)
            nc.vector.memset(ones_row[:], 1.0)
            nc.scalar.activation(act_scr[:], ones_sb[:], AF.Exp)
            Wv_sb = const_p.tile([128, 4, H * D], bf16)

            # ---- per-batch state ----
            qTs = [None, None]
            vTs = [None, None]
            vns = [None, None]
            attnTs = [None, None]
            QTs = []
            VTh = []  # VTh[2*b+h]: [128, 4, 512] = V^T lk-half h of batch b
            for b in range(BPC):
                QTs.append(qt_p.tile([128, 4, LQ], bf16, name="QT",
                                     tag="QT"))
                VTh.append(vt_p.tile([128, 4, LK // 2], bf16, name="VTh",
                                     tag="VTh"))
                VTh.append(vt_p.tile([128, 4, LK // 2], bf16, name="VTh",
                                     tag="VTh"))
                qTs[b] = qproj_p.tile([128, 4, LQ], bf16, name="qT",
                                      tag="qT")
                vTs[b] = vproj_p.tile([128, 4, LK], bf16, name="vT",
                                      tag="vT")
                vns[b] = vn_p.tile([128, 8, 8, 68], bf16, name="vn",
                                   tag="vn")
                attnTs[b] = at_p.tile([128, 4, LQ], bf16, name="attnT",
                                      tag="attnT")
                nc.vector.memset(vns[b][:, :, :, 64:65], 1.0)

            # PE warmup: dummy K=1 matmuls so the HAM clock gate is at
            # 8/8 when the real work arrives
            ps_warm = ps_f.tile([128, 512], f32, name="psf", tag="psf")
            for _ in range(8):
                nc.tensor.matmul(ps_warm[0:64, :], ones_sb[:],
                                 ones_row[:], start=True, stop=True)

            # ---- PE transpose groups: fp32 SBUF -> PSUM -> bf16 SBUF ----
            def t_group_v(b, h, ko, drain=None):
                src = vstg[(b, h)]

                def emit():
                    ps = ps_f.tile([128, 512], f32, name="psf", tag="psf")
                    for j in range(4):
                        nc.tensor.transpose(
                            ps[:, j * 128:(j + 1) * 128],
                            src[:, j, ko * 128:(ko + 1) * 128], ident[:])
                    (drain or _drain_v)(VTh[2 * b + h][:, ko, :], ps[:])
                return emit

            def _drain_v(out, ps):
                nc.vector.tensor_copy(out, ps)

            def _drain_s(out, ps):
                nc.scalar.copy(out, ps)

            def t_group_q(b, ko, drain=_drain_v):
                src = qstg[b]

                def emit():
                    ps = ps_f.tile([128, 512], f32, name="psf", tag="psf")
                    for j in range(4):
                        nc.tensor.transpose(
                            ps[:, j * 128:(j + 1) * 128],
                            src[:, j, ko * 128:(ko + 1) * 128], ident[:])
                    drain(QTs[b][:, ko, :], ps[:])
                return emit

            # ---- projection work units ----
            def unit_vt_proj(b, n, m, drain=None):
                def emit():
                    ps = ps_f.tile([128, 512], f32, name="psf", tag="psf")
                    for kk in range(4):
                        nc.tensor.matmul(
                            ps[:], Wv_sb[:, kk, m * 128:(m + 1) * 128],
                            VTh[2 * b + n][:, kk, :],
                            start=(kk == 0), stop=(kk == 3))
                    (drain or _drain_v)(
                        vTs[b][:, m, n * 512:(n + 1) * 512], ps[:])
                return emit

            def unit_vn_proj(b, c):
                def emit():
                    ps = ps_f.tile([128, 512], f32, name="psf", tag="psf")
                    for kk in range(4):
                        nc.tensor.matmul(
                            ps[:],
                            VTh[2 * b + c // 4][:, kk,
                                                (c % 4) * 128:
                                                (c % 4 + 1) * 128],
                            Wv_sb[:, kk, :], start=(kk == 0), stop=(kk == 3))
                    nc.vector.tensor_copy(
                        vns[b][:, c, :, 0:64],
                        ps[:].rearrange("p (h d) -> p h d", d=64))
                return emit

            def unit_q_proj(b, m, drain=None):
                # bias folded into the drain as a per-partition scalar add
                # (no rank-1 PE matmul)
                def emit():
                    ps = ps_f.tile([128, 512], f32, name="psf", tag="psf")
                    for kk in range(4):
                        nc.tensor.matmul(
                            ps[:], Wq_sb[:, kk, m * 128:(m + 1) * 128],
                            QTs[b][:, kk, :], start=(kk == 0),
                            stop=(kk == 3))
                    nc.vector.tensor_scalar_add(
                        qTs[b][:, m, :], ps[:], bq_f32[:, m:m + 1])
                return emit

            def unit_xbar_v1(h):
                # batch 1 V^T via XBAR DMA-transpose from the cast-staged
                # DRAM copy (sync ring only; scalar-ring xbar corrupts)
                def emit():
                    for ko in range(4):
                        nc.sync.dma_start(
                            VTh[2 + h][:, ko, :],
                            Vbf1[h][:, ko * 128:(ko + 1) * 128],
                            transpose=True)
                return emit

            def unit_xbar_q1():
                def emit():
                    for ko in range(4):
                        nc.sync.dma_start(
                            QTs[1][:, ko, :],
                            Qbf1[:, ko * 128:(ko + 1) * 128],
                            transpose=True)
                return emit

            def fc_q_part(b, m, pool_=None):
                if pool_ is None:
                    ps = ps_f.tile([128, 512], f32, name="psf", tag="psf")
                else:
                    ps = pool_
                # bias row first (start clears the bank)
                nc.tensor.matmul(ps[:], ones_row[:, 0:128], bfp_row[:],
                                 start=True, stop=False)
                for kk in range(4):
                    nc.tensor.matmul(
                        ps[:], QTs[b][:, kk, m * 128:(m + 1) * 128],
                        Wf_sb[:, kk + 4, :], start=False, stop=False)
                return ps

            def fc_attn_partial(b, m, ps, split2=False):
                # pairs 0-2 only: independent of the last pair's 1/S chain
                for kk in range(2):
                    nc.tensor.matmul(
                        ps[:], attnTs[b][:, kk, m * 128:(m + 1) * 128],
                        Wf_sb[:, kk, :], start=False, stop=False)
                if split2:
                    # pair 2 via the odd-tile split (no writeback DMA gate)
                    nc.tensor.matmul(
                        ps[:], attnTs[b][0:64, 2, m * 128:(m + 1) * 128],
                        Wf_sb[0:64, 2, :], start=False, stop=False)
                    nc.tensor.matmul(
                        ps[:], a2o[:, m * 128:(m + 1) * 128],
                        Wf2_odd[:], start=False, stop=False)
                else:
                    nc.tensor.matmul(
                        ps[:], attnTs[b][:, 2, m * 128:(m + 1) * 128],
                        Wf_sb[:, 2, :], start=False, stop=False)

            def fc_attn_final(b, m, ps, ring=None, split=False):
                if split:
                    # kk=3 close as two K=64 matmuls, BOTH at tile position
                    # (0,0): pair 3's odd half lives in a partition-0:64
                    # tile (a3o) against a partition-shifted copy of Wf's
                    # odd rows, so no cross-partition DMA gates the close.
                    nc.tensor.matmul(
                        ps[:], attnTs[b][0:64, 3, m * 128:(m + 1) * 128],
                        Wf_sb[0:64, 3, :], start=False, stop=False)
                    nc.tensor.matmul(
                        ps[:], a3o[:, m * 128:(m + 1) * 128],
                        Wf3_odd[:], start=False, stop=True)
                else:
                    nc.tensor.matmul(
                        ps[:], attnTs[b][:, 3, m * 128:(m + 1) * 128],
                        Wf_sb[:, 3, :], start=False, stop=True)
                osb = osb_p.tile([128, 512], f32, name="osb", tag="osb")
                nc.scalar.activation(osb[:], ps[:], AF.Tanh)
                (ring or nc.sync).dma_start(
                    Od[b][m * 128:(m + 1) * 128, :], osb[:])

            def unit_fc(b, m):
                def emit():
                    ps = fc_q_part(b, m)
                    fc_attn_partial(b, m, ps)
                    fc_attn_final(b, m, ps)
                return emit

            # bf' = bf + bv @ Wf[:H*D]  (the folded value-bias correction)
            bfp_row = const_p.tile([1, F], bf16)
            # tail split-close tiles: pair-2/3 odd attn halves at
            # partitions 0:64 + matching partition-shifted Wf rows
            a2o = const_p.tile([64, 512], bf16)
            Wf2_odd = const_p.tile([64, 512], bf16)
            a3o = const_p.tile([64, 512], bf16)
            Wf3_odd = const_p.tile([64, 512], bf16)

            def emit_bfp():
                ps = ps_f.tile([128, 512], f32, name="psf", tag="psf")
                for k in range(4):
                    nc.tensor.matmul(ps[0:1, :], bv_col[:, k:k + 1],
                                     Wf_sb[:, k, :], start=(k == 0),
                                     stop=(k == 3))
                nc.vector.tensor_tensor(bfp_row[:], ps[0:1, :], bf_row[:],
                                        op=OP.add)

            # ---- attention machinery ----
            # Scores/exp pipeline per head-pair p: per lk-chunk c one psS
            # [128,2,512] (head A bank 0, head B bank 1) -> ONE exp
            # instruction into E[:, :, c, :].  The attn matmuls (M=65 with
            # the ones column) accumulate per head into ps_at, drained to
            # au/s2 by the DVE.  The 1/S chain is pipelined 2 pairs deep.
            def make_stage_B(st):
                st["r8"] = r4_p.tile([16, 64], f32, name="r8", tag="r8")
                nc.vector.reciprocal(st["r8"][:], st["s8"][:])
                st["r8b"] = r4_p.tile([16, 64], bf16, name="r8b", tag="r8b")
                nc.vector.tensor_copy(st["r8b"][:], st["r8"][:])
                st["r2"] = r0_p.tile([1, 2, 512], bf16, name="r2", tag="r2")
                st.get("ring", nc.sync).dma_start(st["r2"][:], st["r8b"][:])

            def make_stage_C(st, attnT, odd_tile=None):
                for s in range(2):
                    psR = ps_f.tile([128, 512], f32, name="psf", tag="psf")
                    nc.tensor.matmul(psR[0:64, :], ones_sb[:],
                                     st["r2"][0:1, s, :],
                                     start=True, stop=True)
                    rbc = rbc_p.tile([64, 512], f32, name="rbc", tag="rbc")
                    nc.vector.tensor_copy(rbc[:], psR[0:64, :])
                    if s == 0:
                        nc.vector.tensor_tensor(
                            attnT[0:64, st["p"], :], st["au"][s][:],
                            rbc[:], op=OP.mult)
                    elif odd_tile is not None:
                        # tail pair: keep the odd half in a partition-0:64
                        # tile (no cross-partition writeback DMA); the fc
                        # close reads it via a split K=64 accumulation
                        nc.vector.tensor_tensor(
                            odd_tile[:], st["au"][s][:], rbc[:],
                            op=OP.mult)
                    else:
                        an = ao_p.tile([64, 512], bf16, name="an", tag="an")
                        nc.vector.tensor_tensor(
                            an[:], st["au"][s][:], rbc[:], op=OP.mult)
                        st.get("ring", nc.sync).dma_start(
                            attnT[64:128, st["p"], :], an[:])

            def attn_batch(b, fill, popn=1, tail_dma=False, tail_ring=None,
                           odd2=None, odd3=None):
                """Emit the attention phase for batch b.  `fill` is a _FillQ
                of PE work-unit closures popped between exp-gated slots;
                ensure() barriers guarantee emission-order correctness for
                units this batch's own stream depends on."""
                qT, vT, vn = qTs[b], vTs[b], vns[b]
                attnT = attnTs[b]
                chain = []

                def scores_chunk(p, c, E):
                    psS = ps_sc.tile([128, 2, 512], f32, name="psS",
                                     tag="sc")
                    nc.tensor.matmul(
                        psS[:, 0, :], vT[0:64, p, c * 128:(c + 1) * 128],
                        qT[0:64, p, :], start=True, stop=True)
                    nc.tensor.matmul(
                        psS[:, 1, :], vT[64:128, p, c * 128:(c + 1) * 128],
                        qT[64:128, p, :], start=True, stop=True)
                    nc.scalar.activation(E[:, c, :, :], psS[:], AF.Exp)

                def attn_head(p, s, E, st):
                    h = 2 * p + s
                    psAt = ps_at.tile([128, 512], f32, name="psAt",
                                      tag="at")
                    for c in range(8):
                        nc.tensor.matmul(
                            psAt[0:65, :], vn[:, c, h, 0:65], E[:, c, s, :],
                            start=(c == 0), stop=(c == 7))
                    nc.vector.tensor_copy(st["s2"][64:65, s, :],
                                          psAt[64:65, :])
                    au = au_p.tile([64, 512], f32, name="au", tag="au")
                    nc.vector.tensor_copy(au[:], psAt[0:64, :])
                    st["au"][s] = au

                Es = []
                sts = []
                fill.ensure(f"t{b}_0")
                # prologue: scores for pair 0
                E0 = e_p.tile([128, 8, 2, 512], bf16, name="E", tag="E")
                for c in range(8):
                    scores_chunk(0, c, E0)
                    if c in (2, 5):
                        fill.pop(popn)
                Es.append(E0)

                for p in range(4):
                    if p < 3:
                        fill.ensure(f"t{b}_{p + 1}")
                    E = Es[p]
                    st = {"p": p, "au": [None, None]}
                    st["s2"] = s_p.tile([65, 2, 512], f32, name="s2",
                                        tag="s2")
                    sts.append(st)
                    if p < 3:
                        En = e_p.tile([128, 8, 2, 512], bf16, name="E",
                                      tag="E")
                        Es.append(En)
                        # interleave next pair's scores with this pair's
                        # attn matmuls; both are exp-gated so filler units
                        # absorb the PE slack.
                        for c in range(8):
                            scores_chunk(p + 1, c, En)
                            if c == 1:
                                fill.ensure(f"vn{b}")
                                attn_head(p, 0, E, st)
                                fill.pop(popn)
                            elif c == 4:
                                attn_head(p, 1, E, st)
                                fill.pop(popn)
                            elif c == 6:
                                fill.pop(popn)
                    else:
                        # last pair: interleave BOTH heads' accumulation
                        # chains (separate ps_at banks).  Sequential heads
                        # make head 1's 7 ready matmuls queue behind head
                        # 0's final-exp-gated chunk-7 matmul, serializing
                        # ~3us after the last exp lands.
                        fill.ensure(f"vn{b}")
                        psA = ps_at.tile([128, 512], f32, name="psAt",
                                         tag="at")
                        psB = ps_at.tile([128, 512], f32, name="psAt",
                                         tag="at")
                        for c in range(8):
                            nc.tensor.matmul(
                                psA[0:65, :], vn[:, c, 2 * p, 0:65],
                                E[:, c, 0, :], start=(c == 0),
                                stop=(c == 7))
                            nc.tensor.matmul(
                                psB[0:65, :], vn[:, c, 2 * p + 1, 0:65],
                                E[:, c, 1, :], start=(c == 0),
                                stop=(c == 7))
                            if c == 2:
                                fill.pop(popn)
                        for s, psX in ((0, psA), (1, psB)):
                            nc.vector.tensor_copy(st["s2"][64:65, s, :],
                                                  psX[64:65, :])
                            au = au_p.tile([64, 512], f32, name="au",
                                           tag="au")
                            nc.vector.tensor_copy(au[:], psX[0:64, :])
                            st["au"][s] = au
                        fill.pop(popn)
                    st["s8"] = s4_p.tile([16, 64], f32, name="s8", tag="s8")
                    ring = tail_ring if (p == 3 and tail_ring) else nc.sync
                    ring.dma_start(st["s8"][:], st["s2"][64:65, :, :])
                    chain.append(st)
                    if len(chain) >= 2:
                        make_stage_B(chain[-2])
                    if len(chain) >= 3:
                        make_stage_C(chain[-3], attnT)
                        fill.pop(popn)
                # flush of the last two pairs' normalize chain is returned
                # as closures so the caller can overlap it with the next
                # batch's prologue (the chain has ~3us of DMA latency).
                def flush_a():
                    # tail-pair DMAs may ride the scalar ring: it is idle
                    # once the exp stream is done, and this unclogs the
                    # sync ring's critical 1/S chain.
                    if tail_ring is not None:
                        chain[-1]["ring"] = tail_ring
                        chain[-2]["ring"] = tail_ring
                    make_stage_B(chain[-1])
                    make_stage_C(chain[-2], attnT, odd_tile=odd2)

                def flush_b():
                    make_stage_C(chain[-1], attnT, odd_tile=odd3)
                return [flush_a, flush_b]

            # ================= emission =================
            # Inline critical prefix: V0 + Q0 transposes, Wq cast, pair-0
            # trio of batch 0 so the first scores fire ~13us in.  The
            # Q0-side PSUM drains go to the (idle until first exp) Scalar
            # engine so the DVE's drain FIFO isn't the startup bottleneck.
            # ALL inline startup drains go to the (idle until first exp)
            # Scalar engine: the Tile scheduler orders the DVE queue by its
            # own cost model and kept head-blocking the drain stream with
            # weight casts, cascading into ~14us of PE idle.  The DVE only
            # does the weight/bias casts here.
            nc.vector.tensor_copy(Wv_sb[:], Wv_f32[:])
            Wq_sb = const_p.tile([128, 4, H * D], bf16)
            nc.vector.tensor_copy(Wq_sb[:], Wq_f32[:])
            for ko in range(4):
                t_group_v(0, 0, ko, drain=_drain_s)()
            for ko in range(4):
                t_group_v(0, 1, ko, drain=_drain_s)()
            for ko in range(4):
                t_group_q(0, ko, drain=_drain_s)()
            unit_vt_proj(0, 0, 0, drain=_drain_s)()
            unit_vt_proj(0, 1, 0, drain=_drain_s)()
            unit_q_proj(0, 0)()

            # Filler queue for batch 0's attention, in need-order.  The
            # xbar units cost no PE (sync-ring issues) so they ride along.
            fill = _FillQ()
            fill.done.add("t0_0")
            for c in range(8):
                fill.push(f"vn0_{c}" if c < 7 else "vn0", unit_vn_proj(0, c))
            for p in range(1, 4):
                fill.push(f"t0_{p}a", unit_vt_proj(0, 0, p))
                fill.push(f"t0_{p}b", unit_vt_proj(0, 1, p))
                fill.push(f"t0_{p}", unit_q_proj(0, p))
            fill.push("xv1h0", unit_xbar_v1(0))
            fill.push("xv1h1", unit_xbar_v1(1))
            fill.push("xq1", unit_xbar_q1())
            fill.push("t1_0a", unit_vt_proj(1, 0, 0))
            fill.push("t1_0b", unit_vt_proj(1, 1, 0))
            fill.push("t1_0", unit_q_proj(1, 0))
            for c in range(8):
                fill.push(f"vn1_{c}" if c < 7 else "vn1", unit_vn_proj(1, c))
            for p in range(1, 4):
                fill.push(f"t1_{p}a", unit_vt_proj(1, 0, p))
                fill.push(f"t1_{p}b", unit_vt_proj(1, 1, p))
                fill.push(f"t1_{p}", unit_q_proj(1, p))

            flush0 = attn_batch(0, fill, popn=2, tail_dma=True)
            fill.drain()

            # batch hinge: bv cast (data long since landed) + bf' fold
            bv_col = const_p.tile([128, 4], bf16)
            nc.vector.tensor_copy(bv_col[:], bv_f32[:])
            emit_bfp()
            # partition-shifted copies of Wf's pair-2/3 odd rows for the
            # tail split-close (one-time SBUF->SBUF DMAs, off the
            # critical path)
            nc.sync.dma_start(Wf2_odd[:], Wf_sb[64:128, 2, :])
            nc.sync.dma_start(Wf3_odd[:], Wf_sb[64:128, 3, :])

            # batch 1 attention; batch 0's normalize flush + fc as filler
            fill1 = _FillQ()
            fill1.done.update({"t1_0", "t1_1", "t1_2", "t1_3", "vn1"})
            fill1.push("fl0", flush0[0])
            fill1.push("fl1", flush0[1])
            for m in range(4):
                fill1.push(f"fc0_{m}", unit_fc(0, m))
            flush1 = attn_batch(1, fill1, popn=1, tail_ring=nc.scalar,
                                odd2=a2o, odd3=a3o)
            fill1.drain()

            # tail: all four fc groups open in the (now free) scores
            # PSUM pool; everything independent of pair 3's 1/S chain
            # (Q parts + attn pairs 0-2) fills the PE while that chain's
            # DMA latency drains, then only the kk=3 closes + tanh remain.
            psAB = ps_sc.tile([128, 2, 512], f32, name="psS", tag="sc")
            psCD = ps_sc.tile([128, 2, 512], f32, name="psS", tag="sc")
            ps0 = fc_q_part(1, 0, psAB[:, 0, :])
            ps1 = fc_q_part(1, 1, psAB[:, 1, :])
            flush1[0]()
            ps2 = fc_q_part(1, 2, psCD[:, 0, :])
            ps3 = fc_q_part(1, 3, psCD[:, 1, :])
            fc_attn_partial(1, 0, ps0, split2=True)
            fc_attn_partial(1, 1, ps1, split2=True)
            flush1[1]()
            fc_attn_partial(1, 2, ps2, split2=True)
            fc_attn_partial(1, 3, ps3, split2=True)
            # tail stores alternate rings (scalar's ring is free once the
            # exp stream is done) so the last ~1MB doesn't serialize.
            fc_attn_final(1, 0, ps0, ring=nc.sync, split=True)
            fc_attn_final(1, 1, ps1, ring=nc.scalar, split=True)
            fc_attn_final(1, 2, ps2, ring=nc.sync, split=True)
            fc_attn_final(1, 3, ps3, ring=nc.scalar, split=True)

    _split_sync_waits(nc, mybir)
    return nc


def _get_nc():
    if "nc" not in _CACHE:
        _CACHE["nc"] = _build()
    return _CACHE["nc"]


def kernel(Q, V, Wq, bq, Wv, bv, Wf, bf, _trace=False):
    from concourse.bass_utils import run_bass_kernel_spmd

    nc = _get_nc()
    Q = np.ascontiguousarray(np.asarray(Q, dtype=np.float32))
    V = np.ascontiguousarray(np.asarray(V, dtype=np.float32))
    shared = {
        "Wq": np.ascontiguousarray(np.asarray(Wq, np.float32)),
        "bq": np.ascontiguousarray(np.asarray(bq, np.float32)),
        "Wv": np.ascontiguousarray(np.asarray(Wv, np.float32)),
        "bv": np.ascontiguousarray(np.asarray(bv, np.float32)),
        "Wf": np.ascontiguousarray(np.asarray(Wf, np.float32)),
        "bf": np.ascontiguousarray(np.asarray(bf, np.float32)),
    }
    in_maps = []
    for c in range(NCORES):
        m = {"Q": Q[c * BPC:(c + 1) * BPC], "V": V[c * BPC:(c + 1) * BPC]}
        m.update(shared)
        in_maps.append(m)

    res = run_bass_kernel_spmd(nc, in_maps, core_ids=list(range(NCORES)),
                               trace=_trace)
    out = np.concatenate([res.results[c]["O"] for c in range(NCORES)], axis=0)
    if _trace:
        _CACHE["last_exec_time_ns"] = res.exec_time_ns
        _CACHE["last_res"] = res
    return out


# revision 107
# speedup vs baseline: 1.0404x; 1.0404x over previous
"""Trainium2 Bass kernel for nn_MultiHeadAttention_45019847196962.

Reference computation (per batch b):
    q = Q @ Wq + bq                 # (Lq, H*D)
    v = V @ Wv + bv                 # (Lk, H*D)   (used as both keys and values)
    scores = q_h @ v_h^T            # per head, no 1/sqrt(d) scale
    align  = softmax(scores, -1)
    attn   = align @ v_h            # concat heads -> (Lq, H*D)
    out    = tanh([attn | Q] @ Wf + bf)

Sharding: data-parallel over batch. 16 batches / 8 cores = 2 batches per
core; weights replicated. No collectives.

Key algebraic restructuring vs the obvious dataflow:
  - bv is dropped on-device entirely: softmax rows are shift-invariant so
    dropping bv from the keys changes nothing, and align rows sum to one
    so align@(v+bv) = align@v + bv, which folds into the fc bias as
    bf' = bf + bv @ Wf[:H*D].  This removes all value-bias DVE work.
  - bq and bf' are added via K=1 rank-1 accumulation matmuls into the
    same PSUM group as the projection / fc, so no separate bias pass.
  - The softmax denominator S rides as a 65th "ones" column on the attn
    matmul stationary (align rows summing to 1 make this exact).
  - attn normalization (x 1/S) happens on the DVE as a tensor-tensor
    multiply against a PE-broadcast r row, fused with the PSUM drain.

Input staging (v2): Q/V/weights are loaded as PLAIN fp32 DMAs spread
across all three DMA rings (sync + scalar HWDGE, gpsimd SWDGE), then
Q/V are transposed ON the PE via identity matmuls (fp32, 2 cyc/row)
into PSUM, and the PSUM drain doubles as the fp32->bf16 cast on the
DVE.  This replaces the previous serial gpsimd cast-DMA queue + XBAR
DMA-transpose pipeline which kept the PE starved for the first ~60us.
Batch 0's pair-0 projection trio is emitted inline right after its
transposes so the first exp fires ~11us in; everything else (rest of
b0's projections, b1's transposes + projections, b0's fc) flows
through an ordered filler queue popped between exp-gated attention
slots, with ensure() barriers guaranteeing emission order (Tile deps
follow program order, so a consumer must be emitted after its
producer units).

HW facts this kernel leans on (verified by trace):
  - Plain HWDGE/SWDGE DMAs run ~250-320GB/s per ring; dtype-casting
    DMAs and XBAR transposes are far slower / serialize on one ring.
  - PE transpose matmuls cost ~2 cycles/row fp32 (~107ns per 128x128
    tile warm), nearly free in the startup window where the PE idles.
  - exp on the Scalar engine is ~1.15us per [128,2,512] slab; 64 slabs
    = ~73us/core, slightly under the PE's total work, so the kernel is
    PE-bound and the scheduling goal is a dense PE stream.
  - The PE HAM clock-gate needs ~3.4us of activity to reach full
    clock: a short dummy-matmul warmup covers the initial DMA phase.
  - Tiny multi-partition DMAs (<=32B/partition) take ~14us to signal
    completion; bv's load is issued early but its cast is emitted at
    the batch hinge, and the 1/S gather uses a [16,64] shape.
"""

import numpy as np

B, LQ, LK = 16, 512, 1024
F, H, D = 512, 8, 64
NCORES = 8
BPC = B // NCORES  # batches per core

_CACHE = {}


def _split_sync_waits(nc, mybir, maxw=1):
    """This container's walrus rejects instructions with more than one sync
    wait ("Too many sync wait commands").  Move excess waits onto NoOp
    instructions inserted just before the over-subscribed instruction on the
    same engine queue (program order preserves the wait semantics)."""
    for fn in nc.m.functions:
        for blk in fn.blocks:
            insts = blk.instructions
            i = 0
            while i < len(insts):
                inst = insts[i]
                si = getattr(inst, "sync_info", None)
                if si is not None and len(si.on_wait) > maxw:
                    waits = list(si.on_wait)
                    del si.on_wait[maxw:]
                    pre = []
                    for j in range(maxw, len(waits), maxw):
                        nop = mybir.InstNoOp(
                            name=nc.get_next_instruction_name(),
                            engine=inst.engine,
                            ins=[],
                            outs=[],
                            sync_info=mybir.SyncInfo(
                                on_wait=waits[j:j + maxw], on_update=[]),
                        )
                        pre.append(nop)
                    insts[i:i] = pre
                    i += len(pre)
                i += 1


def _patch_sem_clear_chunking(bass, chunk=16):
    """walrus here rejects the kernel-tail SEM_RANGE_CLEAR ISA op when the
    semaphore range is large ("ISA wrong length").  Chunk the ranges."""
    if getattr(bass.Bass.clear_and_free_semaphores, "_chunked", False):
        return
    orig = bass.Bass.clear_and_free_semaphores

    def chunked(self, sems):
        sems = list(sems)
        nums = [s.num if hasattr(s, "num") else s for s in sems]
        order = sorted(range(len(sems)), key=lambda i: nums[i])
        for j in range(0, len(sems), chunk):
            orig(self, [sems[i] for i in order[j:j + chunk]])

    chunked._chunked = True
    bass.Bass.clear_and_free_semaphores = chunked


class _FillQ:
    """Ordered filler-unit queue.  Units are (name, closure) emitted in
    FIFO order; ensure() pops until the named unit has been emitted, which
    makes consumer emission safe regardless of pop tuning (Tile deps follow
    program order)."""

    def __init__(self):
        self.q = []
        self.done = set()

    def push(self, name, fn):
        self.q.append((name, fn))

    def pop(self, k=1):
        for _ in range(k):
            if not self.q:
                return
            name, fn = self.q.pop(0)
            fn()
            self.done.add(name)

    def ensure(self, *names):
        while self.q and any(n not in self.done for n in names):
            self.pop(1)

    def drain(self):
        self.pop(len(self.q))


def _build():
    import concourse.bass as bass
    import concourse.tile as tile
    from concourse import mybir
    from concourse import masks

    _patch_sem_clear_chunking(bass)

    dt = mybir.dt
    f32, bf16 = dt.float32, dt.bfloat16
    AF = mybir.ActivationFunctionType
    OP = mybir.AluOpType

    nc = bass.Bass("TRN2", target_bir_lowering=False, debug=False,
                   num_devices=NCORES)

    Qd = nc.dram_tensor("Q", [BPC, LQ, F], f32, kind="ExternalInput").ap()
    Vd = nc.dram_tensor("V", [BPC, LK, F], f32, kind="ExternalInput").ap()
    Wqd = nc.dram_tensor("Wq", [F, H * D], f32, kind="ExternalInput").ap()
    bqd = nc.dram_tensor("bq", [H * D], f32, kind="ExternalInput").ap()
    Wvd = nc.dram_tensor("Wv", [F, H * D], f32, kind="ExternalInput").ap()
    bvd = nc.dram_tensor("bv", [H * D], f32, kind="ExternalInput").ap()
    Wfd = nc.dram_tensor("Wf", [F + H * D, F], f32, kind="ExternalInput").ap()
    bfd = nc.dram_tensor("bf", [F], f32, kind="ExternalInput").ap()
    Od = nc.dram_tensor("O", [BPC, LQ, F], f32, kind="ExternalOutput").ap()

    with tile.TileContext(nc) as tc:
        import contextlib
        with contextlib.ExitStack() as ctx:
            def pool(name, bufs, space="SBUF"):
                return ctx.enter_context(
                    tc.tile_pool(name=name, bufs=bufs, space=space))

            const_p = pool("const", 1)
            qt_p = pool("qt", 2)        # Q^T bf16 (from PE transpose)
            vt_p = pool("vt", 4)        # V^T, one tile per (batch, lk-half)
            qproj_p = pool("qproj", 2)  # qT
            vproj_p = pool("vproj", 2)  # vT
            vn_p = pool("vn", 2)        # v natural (+ones col)
            e_p = pool("E", 2)          # exp(scores^T) per pair
            at_p = pool("attnT", 2)
            s_p = pool("s_sb", 3)
            au_p = pool("au", 4)
            s4_p = pool("s4", 3)
            r4_p = pool("r4", 3)
            r0_p = pool("r0", 3)
            rbc_p = pool("rbc", 2)
            ao_p = pool("anodd", 1)
            osb_p = pool("osb", 2)

            # fp32 input staging (plain DMAs, cast happens in PSUM drain)
            vstg_p = pool("vstg", 2)    # V halves [128,4,512] f32
            qstg_p = pool("qstg", 1)    # Q batches [128,4,512] f32
            wst_p = pool("wstage", 2)   # Wq/Wv then Wf halves, f32

            # PSUM: 8 banks of [128, 512] f32.
            ps_sc = pool("ps_sc", 2, space="PSUM")  # scores [128,2,512]: 4
            ps_at = pool("ps_at", 2, space="PSUM")  # attn out [128,512]: 2
            ps_f = pool("ps_f", 2, space="PSUM")    # proj/fc/transpose: 2

            # ---- DMA issues, ordered by need per ring ----
            # identity for PE transposes FIRST on the gpsimd engine (no
            # deps, so Tile can't push it behind a blocked DMA issue)
            ident = const_p.tile([128, 128], f32)
            masks.make_identity(nc, ident[:])
            # Wq rides the gpsimd ring ahead of the batch-1 cast-DMAs:
            # it lands ~17us (vs ~21 as the 4th sync-ring load) and the
            # cast-DMAs then queue behind it on the same FIFO.
            Wq_f32 = wst_p.tile([128, 4, H * D], f32, name="wstage",
                                tag="wstage")
            nc.gpsimd.dma_start(
                Wq_f32[:], Wqd.rearrange("(ko p) n -> p ko n", p=128))

            vstg = {}
            qstg = [None]

            def v_load(engine, b, h):
                t = vstg_p.tile([128, 4, 512], f32, name="vstg", tag="vstg")
                engine.dma_start(
                    t[:],
                    Vd[b][h * 512:(h + 1) * 512].rearrange(
                        "(ro p) f -> p ro f", p=128))
                vstg[(b, h)] = t

            # Batch 0 inputs + Wq/Wv: plain fp32 loads on the two HWDGE
            # rings (V0/Q0 then PE-transposed; weights DVE-cast).
            v_load(nc.sync, 0, 0)     # sync ring
            v_load(nc.scalar, 0, 1)   # scalar ring
            qstg[0] = qstg_p.tile([128, 4, 512], f32, name="qstg",
                                  tag="qstg")
            nc.sync.dma_start(
                qstg[0][:], Qd[0].rearrange("(ro p) f -> p ro f", p=128))
            Wv_f32 = wst_p.tile([128, 4, H * D], f32, name="wstage",
                                tag="wstage")
            nc.scalar.dma_start(
                Wv_f32[:], Wvd.rearrange("(ko p) n -> p ko n", p=128))

            # bq in qT layout [128,4] (per-partition bias column) via a
            # tiny gpsimd DMA -- the ~14us completion-signal latency is
            # fine since it's needed only at the first q drain (~28us).
            bq_f32 = const_p.tile([128, 4], f32)
            nc.gpsimd.dma_start(
                bq_f32[:], bqd.rearrange("(ko p) -> p ko", p=128))
            # bf/bv: tiny multi-partition DMAs with multi-us descriptor-gen
            # cost -- keep them off the busy HWDGE rings (gpsimd instead).
            bf_row = const_p.tile([1, F], f32)
            nc.gpsimd.dma_start(bf_row[:],
                                bfd.rearrange("(a n) -> a n", a=1))
            bv_f32 = const_p.tile([128, 4], f32)
            nc.gpsimd.dma_start(
                bv_f32[:], bvd.rearrange("(ko p) -> p ko", p=128))

            # Batch 1 inputs + Wf: fp32->bf16 cast-DMAs on the async SWDGE
            # queue (none needed before ~45us; zero PE/DVE cost).  Their
            # transfers hog the DMA fabric and stall the HWDGE rings, so a
            # tiny gpsimd op that depends on the LAST critical plain load
            # (Wq) fences their issue until the startup loads are done.
            gp_scr = const_p.tile([1, 64], f32)
            nc.gpsimd.tensor_copy(gp_scr[:], Wq_f32[0:1, 0, 0:64])
            Vbf1 = nc.dram_tensor("Vbf1", [2, LK // 2, F], bf16).ap()
            Qbf1 = nc.dram_tensor("Qbf1", [LQ, F], bf16).ap()
            nc.gpsimd.dma_start(Vbf1[0], Vd[1][0:512])
            nc.gpsimd.dma_start(Vbf1[1], Vd[1][512:1024])
            nc.gpsimd.dma_start(Qbf1[:], Qd[1])
            Wf_sb = const_p.tile([128, 8, F], bf16)
            for wh in range(2):
                nc.gpsimd.dma_start(
                    Wf_sb[:, wh * 4:(wh + 1) * 4, :],
                    Wfd[wh * 512:(wh + 1) * 512].rearrange(
                        "(ko p) n -> p ko n", p=128))

            # preload the activation table set (exp+tanh) while the input
            # DMAs are in flight so the first real exp doesn't pay ~2.2us
            act_scr = const_p.tile([1, 64], f32)

            # ---- DVE constants / casts (ordered by data arrival) ----
            ones_sb = const_p.tile([1, 64], bf16)
            nc.vector.memset(ones_sb[:], 1.0)
            ones_row = const_p.tile([1, F], bf16)
            nc.vector.memset(ones_row[:], 1.0)
            nc.scalar.activation(act_scr[:], ones_sb[:], AF.Exp)
            Wv_sb = const_p.tile([128, 4, H * D], bf16)

            # ---- per-batch state ----
            qTs = [None, None]
            vTs = [None, None]
            vns = [None, None]
            attnTs = [None, None]
            QTs = []
            VTh = []  # VTh[2*b+h]: [128, 4, 512] = V^T lk-half h of batch b
            for b in range(BPC):
                QTs.append(qt_p.tile([128, 4, LQ], bf16, name="QT",
                                     tag="QT"))
                VTh.append(vt_p.tile([128, 4, LK // 2], bf16, name="VTh",
                                     tag="VTh"))
                VTh.append(vt_p.tile([128, 4, LK // 2], bf16, name="VTh",
                                     tag="VTh"))
                qTs[b] = qproj_p.tile([128, 4, LQ], bf16, name="qT",
                                      tag="qT")
                vTs[b] = vproj_p.tile([128, 4, LK], bf16, name="vT",
                                      tag="vT")
                vns[b] = vn_p.tile([128, 8, 8, 68], bf16, name="vn",
                                   tag="vn")
                attnTs[b] = at_p.tile([128, 4, LQ], bf16, name="attnT",
                                      tag="attnT")
                nc.vector.memset(vns[b][:, :, :, 64:65], 1.0)

            # PE warmup: dummy K=1 matmuls so the HAM clock gate is at
            # 8/8 when the real work arrives
            ps_warm = ps_f.tile([128, 512], f32, name="psf", tag="psf")
            for _ in range(8):
                nc.tensor.matmul(ps_warm[0:64, :], ones_sb[:],
                                 ones_row[:], start=True, stop=True)

            # ---- PE transpose groups: fp32 SBUF -> PSUM -> bf16 SBUF ----
            def t_group_v(b, h, ko, drain=None):
                src = vstg[(b, h)]

                def emit():
                    ps = ps_f.tile([128, 512], f32, name="psf", tag="psf")
                    for j in range(4):
                        nc.tensor.transpose(
                            ps[:, j * 128:(j + 1) * 128],
                            src[:, j, ko * 128:(ko + 1) * 128], ident[:])
                    (drain or _drain_v)(VTh[2 * b + h][:, ko, :], ps[:])
                return emit

            def _drain_v(out, ps):
                nc.vector.tensor_copy(out, ps)

            def _drain_s(out, ps):
                nc.scalar.copy(out, ps)

            def t_group_q(b, ko, drain=_drain_v):
                src = qstg[b]

                def emit():
                    ps = ps_f.tile([128, 512], f32, name="psf", tag="psf")
                    for j in range(4):
                        nc.tensor.transpose(
                            ps[:, j * 128:(j + 1) * 128],
                            src[:, j, ko * 128:(ko + 1) * 128], ident[:])
                    drain(QTs[b][:, ko, :], ps[:])
                return emit

            # ---- projection work units ----
            def unit_vt_proj(b, n, m, drain=None):
                def emit():
                    ps = ps_f.tile([128, 512], f32, name="psf", tag="psf")
                    for kk in range(4):
                        nc.tensor.matmul(
                            ps[:], Wv_sb[:, kk, m * 128:(m + 1) * 128],
                            VTh[2 * b + n][:, kk, :],
                            start=(kk == 0), stop=(kk == 3))
                    (drain or _drain_v)(
                        vTs[b][:, m, n * 512:(n + 1) * 512], ps[:])
                return emit

            def unit_vn_proj(b, c):
                def emit():
                    ps = ps_f.tile([128, 512], f32, name="psf", tag="psf")
                    for kk in range(4):
                        nc.tensor.matmul(
                            ps[:],
                            VTh[2 * b + c // 4][:, kk,
                                                (c % 4) * 128:
                                                (c % 4 + 1) * 128],
                            Wv_sb[:, kk, :], start=(kk == 0), stop=(kk == 3))
                    nc.vector.tensor_copy(
                        vns[b][:, c, :, 0:64],
                        ps[:].rearrange("p (h d) -> p h d", d=64))
                return emit

            def unit_q_proj(b, m, drain=None):
                # bias folded into the drain as a per-partition scalar add
                # (no rank-1 PE matmul)
                def emit():
                    ps = ps_f.tile([128, 512], f32, name="psf", tag="psf")
                    for kk in range(4):
                        nc.tensor.matmul(
                            ps[:], Wq_sb[:, kk, m * 128:(m + 1) * 128],
                            QTs[b][:, kk, :], start=(kk == 0),
                            stop=(kk == 3))
                    nc.vector.tensor_scalar_add(
                        qTs[b][:, m, :], ps[:], bq_f32[:, m:m + 1])
                return emit

            def unit_xbar_v1(h):
                # batch 1 V^T via XBAR DMA-transpose from the cast-staged
                # DRAM copy (sync ring only; scalar-ring xbar corrupts)
                def emit():
                    for ko in range(4):
                        nc.sync.dma_start(
                            VTh[2 + h][:, ko, :],
                            Vbf1[h][:, ko * 128:(ko + 1) * 128],
                            transpose=True)
                return emit

            def unit_xbar_q1():
                def emit():
                    for ko in range(4):
                        nc.sync.dma_start(
                            QTs[1][:, ko, :],
                            Qbf1[:, ko * 128:(ko + 1) * 128],
                            transpose=True)
                return emit

            def fc_q_part(b, m, pool_=None):
                if pool_ is None:
                    ps = ps_f.tile([128, 512], f32, name="psf", tag="psf")
                else:
                    ps = pool_
                # bias row first (start clears the bank)
                nc.tensor.matmul(ps[:], ones_row[:, 0:128], bfp_row[:],
                                 start=True, stop=False)
                for kk in range(4):
                    nc.tensor.matmul(
                        ps[:], QTs[b][:, kk, m * 128:(m + 1) * 128],
                        Wf_sb[:, kk + 4, :], start=False, stop=False)
                return ps

            def fc_attn_partial(b, m, ps, split2=False):
                # pairs 0-2 only: independent of the last pair's 1/S chain
                for kk in range(2):
                    nc.tensor.matmul(
                        ps[:], attnTs[b][:, kk, m * 128:(m + 1) * 128],
                        Wf_sb[:, kk, :], start=False, stop=False)
                if split2:
                    # pair 2 via the odd-tile split (no writeback DMA gate)
                    nc.tensor.matmul(
                        ps[:], attnTs[b][0:64, 2, m * 128:(m + 1) * 128],
                        Wf_sb[0:64, 2, :], start=False, stop=False)
                    nc.tensor.matmul(
                        ps[:], a2o[:, m * 128:(m + 1) * 128],
                        Wf2_odd[:], start=False, stop=False)
                else:
                    nc.tensor.matmul(
                        ps[:], attnTs[b][:, 2, m * 128:(m + 1) * 128],
                        Wf_sb[:, 2, :], start=False, stop=False)

            def fc_attn_final(b, m, ps, ring=None, split=False):
                if split:
                    # kk=3 close as two K=64 matmuls, BOTH at tile position
                    # (0,0): pair 3's odd half lives in a partition-0:64
                    # tile (a3o) against a partition-shifted copy of Wf's
                    # odd rows, so no cross-partition DMA gates the close.
                    nc.tensor.matmul(
                        ps[:], attnTs[b][0:64, 3, m * 128:(m + 1) * 128],
                        Wf_sb[0:64, 3, :], start=False, stop=False)
                    nc.tensor.matmul(
                        ps[:], a3o[:, m * 128:(m + 1) * 128],
                        Wf3_odd[:], start=False, stop=True)
                else:
                    nc.tensor.matmul(
                        ps[:], attnTs[b][:, 3, m * 128:(m + 1) * 128],
                        Wf_sb[:, 3, :], start=False, stop=True)
                osb = osb_p.tile([128, 512], f32, name="osb", tag="osb")
                nc.scalar.activation(osb[:], ps[:], AF.Tanh)
                (ring or nc.sync).dma_start(
                    Od[b][m * 128:(m + 1) * 128, :], osb[:])

            def unit_fc(b, m):
                def emit():
                    ps = fc_q_part(b, m)
                    fc_attn_partial(b, m, ps)
                    fc_attn_final(b, m, ps)
                return emit

            # bf' = bf + bv @ Wf[:H*D]  (the folded value-bias correction)
            bfp_row = const_p.tile([1, F], bf16)
            # tail split-close tiles: pair-2/3 odd attn halves at
            # partitions 0:64 + matching partition-shifted Wf rows
            a2o = const_p.tile([64, 512], bf16)
            Wf2_odd = const_p.tile([64, 512], bf16)
            a3o = const_p.tile([64, 512], bf16)
            Wf3_odd = const_p.tile([64, 512], bf16)

            def emit_bfp():
                ps = ps_f.tile([128, 512], f32, name="psf", tag="psf")
                for k in range(4):
                    nc.tensor.matmul(ps[0:1, :], bv_col[:, k:k + 1],
                                     Wf_sb[:, k, :], start=(k == 0),
                                     stop=(k == 3))
                nc.vector.tensor_tensor(bfp_row[:], ps[0:1, :], bf_row[:],
                                        op=OP.add)

            # ---- attention machinery ----
            # Scores/exp pipeline per head-pair p: per lk-chunk c one psS
            # [128,2,512] (head A bank 0, head B bank 1) -> ONE exp
            # instruction into E[:, :, c, :].  The attn matmuls (M=65 with
            # the ones column) accumulate per head into ps_at, drained to
            # au/s2 by the DVE.  The 1/S chain is pipelined 2 pairs deep.
            def make_stage_B(st):
                st["r8"] = r4_p.tile([16, 64], f32, name="r8", tag="r8")
                nc.vector.reciprocal(st["r8"][:], st["s8"][:])
                st["r8b"] = r4_p.tile([16, 64], bf16, name="r8b", tag="r8b")
                nc.vector.tensor_copy(st["r8b"][:], st["r8"][:])
                st["r2"] = r0_p.tile([1, 2, 512], bf16, name="r2", tag="r2")
                st.get("ring", nc.sync).dma_start(st["r2"][:], st["r8b"][:])

            def make_stage_C(st, attnT, odd_tile=None):
                for s in range(2):
                    psR = ps_f.tile([128, 512], f32, name="psf", tag="psf")
                    nc.tensor.matmul(psR[0:64, :], ones_sb[:],
                                     st["r2"][0:1, s, :],
                                     start=True, stop=True)
                    rbc = rbc_p.tile([64, 512], f32, name="rbc", tag="rbc")
                    nc.vector.tensor_copy(rbc[:], psR[0:64, :])
                    if s == 0:
                        nc.vector.tensor_tensor(
                            attnT[0:64, st["p"], :], st["au"][s][:],
                            rbc[:], op=OP.mult)
                    elif odd_tile is not None:
                        # tail pair: keep the odd half in a partition-0:64
                        # tile (no cross-partition writeback DMA); the fc
                        # close reads it via a split K=64 accumulation
                        nc.vector.tensor_tensor(
                            odd_tile[:], st["au"][s][:], rbc[:],
                            op=OP.mult)
                    else:
                        an = ao_p.tile([64, 512], bf16, name="an", tag="an")
                        nc.vector.tensor_tensor(
                            an[:], st["au"][s][:], rbc[:], op=OP.mult)
                        st.get("ring", nc.sync).dma_start(
                            attnT[64:128, st["p"], :], an[:])

            def attn_batch(b, fill, popn=1, tail_dma=False, tail_ring=None,
                           odd2=None, odd3=None):
                """Emit the attention phase for batch b.  `fill` is a _FillQ
                of PE work-unit closures popped between exp-gated slots;
                ensure() barriers guarantee emission-order correctness for
                units this batch's own stream depends on."""
                qT, vT, vn = qTs[b], vTs[b], vns[b]
                attnT = attnTs[b]
                chain = []

                def scores_chunk(p, c, E):
                    psS = ps_sc.tile([128, 2, 512], f32, name="psS",
                                     tag="sc")
                    nc.tensor.matmul(
                        psS[:, 0, :], vT[0:64, p, c * 128:(c + 1) * 128],
                        qT[0:64, p, :], start=True, stop=True)
                    nc.tensor.matmul(
                        psS[:, 1, :], vT[64:128, p, c * 128:(c + 1) * 128],
                        qT[64:128, p, :], start=True, stop=True)
                    nc.scalar.activation(E[:, c, :, :], psS[:], AF.Exp)

                def attn_head(p, s, E, st):
                    h = 2 * p + s
                    psAt = ps_at.tile([128, 512], f32, name="psAt",
                                      tag="at")
                    for c in range(8):
                        nc.tensor.matmul(
                            psAt[0:65, :], vn[:, c, h, 0:65], E[:, c, s, :],
                            start=(c == 0), stop=(c == 7))
                    nc.vector.tensor_copy(st["s2"][64:65, s, :],
                                          psAt[64:65, :])
                    au = au_p.tile([64, 512], f32, name="au", tag="au")
                    nc.vector.tensor_copy(au[:], psAt[0:64, :])
                    st["au"][s] = au

                Es = []
                sts = []
                fill.ensure(f"t{b}_0")
                # prologue: scores for pair 0
                E0 = e_p.tile([128, 8, 2, 512], bf16, name="E", tag="E")
                for c in range(8):
                    scores_chunk(0, c, E0)
                    if c in (2, 5):
                        fill.pop(popn)
                Es.append(E0)

                for p in range(4):
                    if p < 3:
                        fill.ensure(f"t{b}_{p + 1}")
                    E = Es[p]
                    st = {"p": p, "au": [None, None]}
                    st["s2"] = s_p.tile([65, 2, 512], f32, name="s2",
                                        tag="s2")
                    sts.append(st)
                    if p < 3:
                        En = e_p.tile([128, 8, 2, 512], bf16, name="E",
                                      tag="E")
                        Es.append(En)
                        # interleave next pair's scores with this pair's
                        # attn matmuls; both are exp-gated so filler units
                        # absorb the PE slack.
                        for c in range(8):
                            scores_chunk(p + 1, c, En)
                            if c == 1:
                                fill.ensure(f"vn{b}")
                                attn_head(p, 0, E, st)
                                fill.pop(popn)
                            elif c == 4:
                                attn_head(p, 1, E, st)
                                fill.pop(popn)
                            elif c == 6:
                                fill.pop(popn)
                    else:
                        # last pair: interleave BOTH heads' accumulation
                        # chains (separate ps_at banks).  Sequential heads
                        # make head 1's 7 ready matmuls queue behind head
                        # 0's final-exp-gated chunk-7 matmul, serializing
                        # ~3us after the last exp lands.
                        fill.ensure(f"vn{b}")
                        psA = ps_at.tile([128, 512], f32, name="psAt",
                                         tag="at")
                        psB = ps_at.tile([128, 512], f32, name="psAt",
                                         tag="at")
                        for c in range(8):
                            nc.tensor.matmul(
                                psA[0:65, :], vn[:, c, 2 * p, 0:65],
                                E[:, c, 0, :], start=(c == 0),
                                stop=(c == 7))
                            nc.tensor.matmul(
                                psB[0:65, :], vn[:, c, 2 * p + 1, 0:65],
                                E[:, c, 1, :], start=(c == 0),
                                stop=(c == 7))
                            if c == 2:
                                fill.pop(popn)
                        for s, psX in ((0, psA), (1, psB)):
                            nc.vector.tensor_copy(st["s2"][64:65, s, :],
                                                  psX[64:65, :])
                            au = au_p.tile([64, 512], f32, name="au",
                                           tag="au")
                            nc.vector.tensor_copy(au[:], psX[0:64, :])
                            st["au"][s] = au
                        fill.pop(popn)
                    st["s8"] = s4_p.tile([16, 64], f32, name="s8", tag="s8")
                    ring = tail_ring if (p == 3 and tail_ring) else nc.sync
                    ring.dma_start(st["s8"][:], st["s2"][64:65, :, :])
                    chain.append(st)
                    if len(chain) >= 2:
                        make_stage_B(chain[-2])
                    if len(chain) >= 3:
                        make_stage_C(chain[-3], attnT)
                        fill.pop(popn)
                # flush of the last two pairs' normalize chain is returned
                # as closures so the caller can overlap it with the next
                # batch's prologue (the chain has ~3us of DMA latency).
                def flush_a():
                    # tail-pair DMAs may ride the scalar ring: it is idle
                    # once the exp stream is done, and this unclogs the
                    # sync ring's critical 1/S chain.
                    if tail_ring is not None:
                        chain[-1]["ring"] = tail_ring
                        chain[-2]["ring"] = tail_ring
                    make_stage_B(chain[-1])
                    make_stage_C(chain[-2], attnT, odd_tile=odd2)

                def flush_b():
                    make_stage_C(chain[-1], attnT, odd_tile=odd3)
                return [flush_a, flush_b]

            # ================= emission =================
            # Inline critical prefix: V0 + Q0 transposes, Wq cast, pair-0
            # trio of batch 0 so the first scores fire ~13us in.  The
            # Q0-side PSUM drains go to the (idle until first exp) Scalar
            # engine so the DVE's drain FIFO isn't the startup bottleneck.
            # ALL inline startup drains go to the (idle until first exp)
            # Scalar engine: the Tile scheduler orders the DVE queue by its
            # own cost model and kept head-blocking the drain stream with
            # weight casts, cascading into ~14us of PE idle.  The DVE only
            # does the weight/bias casts here.
            nc.vector.tensor_copy(Wv_sb[:], Wv_f32[:])
            Wq_sb = const_p.tile([128, 4, H * D], bf16)
            nc.vector.tensor_copy(Wq_sb[:], Wq_f32[:])
            for ko in range(4):
                t_group_v(0, 0, ko, drain=_drain_s)()
            for ko in range(4):
                t_group_v(0, 1, ko, drain=_drain_s)()
            for ko in range(4):
                t_group_q(0, ko, drain=_drain_s)()
            unit_vt_proj(0, 0, 0, drain=_drain_s)()
            unit_vt_proj(0, 1, 0, drain=_drain_s)()
            unit_q_proj(0, 0)()

            # Filler queue for batch 0's attention, in need-order.  The
            # xbar units cost no PE (sync-ring issues) so they ride along.
            fill = _FillQ()
            fill.done.add("t0_0")
            for c in range(8):
                fill.push(f"vn0_{c}" if c < 7 else "vn0", unit_vn_proj(0, c))
            for p in range(1, 4):
                fill.push(f"t0_{p}a", unit_vt_proj(0, 0, p))
                fill.push(f"t0_{p}b", unit_vt_proj(0, 1, p))
                fill.push(f"t0_{p}", unit_q_proj(0, p))
            fill.push("xv1h0", unit_xbar_v1(0))
            fill.push("xv1h1", unit_xbar_v1(1))
            fill.push("xq1", unit_xbar_q1())
            fill.push("t1_0a", unit_vt_proj(1, 0, 0))
            fill.push("t1_0b", unit_vt_proj(1, 1, 0))
            fill.push("t1_0", unit_q_proj(1, 0))
            for c in range(8):
                fill.push(f"vn1_{c}" if c < 7 else "vn1", unit_vn_proj(1, c))
            for p in range(1, 4):
                fill.push(f"t1_{p}a", unit_vt_proj(1, 0, p))
                fill.push(f"t1_{p}b", unit_vt_proj(1, 1, p))
                fill.push(f"t1_{p}", unit_q_proj(1, p))

            flush0 = attn_batch(0, fill, popn=2, tail_dma=True)
            fill.drain()

            # batch hinge: bv cast (data long since landed) + bf' fold
            bv_col = const_p.tile([128, 4], bf16)
            nc.vector.tensor_copy(bv_col[:], bv_f32[:])
            emit_bfp()
            # partition-shifted copies of Wf's pair-2/3 odd rows for the
            # tail split-close (one-time SBUF->SBUF DMAs, off the
            # critical path)
            nc.sync.dma_start(Wf2_odd[:], Wf_sb[64:128, 2, :])
            nc.sync.dma_start(Wf3_odd[:], Wf_sb[64:128, 3, :])

            # batch 1 attention; batch 0's normalize flush + fc as filler
            fill1 = _FillQ()
            fill1.done.update({"t1_0", "t1_1", "t1_2", "t1_3", "vn1"})
            fill1.push("fl0", flush0[0])
            fill1.push("fl1", flush0[1])
            for m in range(4):
                fill1.push(f"fc0_{m}", unit_fc(0, m))
            flush1 = attn_batch(1, fill1, popn=1, tail_ring=nc.scalar,
                                odd2=a2o, odd3=a3o)
            fill1.drain()

            # tail: all four fc groups open in the (now free) scores
            # PSUM pool; everything independent of pair 3's 1/S chain
            # (Q parts + attn pairs 0-2) fills the PE while that chain's
            # DMA latency drains, then only the kk=3 closes + tanh remain.
            psAB = ps_sc.tile([128, 2, 512], f32, name="psS", tag="sc")
            psCD = ps_sc.tile([128, 2, 512], f32, name="psS", tag="sc")
            ps0 = fc_q_part(1, 0, psAB[:, 0, :])
            ps1 = fc_q_part(1, 1, psAB[:, 1, :])
            flush1[0]()
            ps2 = fc_q_part(1, 2, psCD[:, 0, :])
            ps3 = fc_q_part(1, 3, psCD[:, 1, :])
            fc_attn_partial(1, 0, ps0, split2=True)
            fc_attn_partial(1, 1, ps1, split2=True)
            flush1[1]()
            fc_attn_partial(1, 2, ps2, split2=True)
            fc_attn_partial(1, 3, ps3, split2=True)
            # tail stores alternate rings (scalar's ring is free once the
            # exp stream is done) so the last ~1MB doesn't serialize.
            fc_attn_final(1, 0, ps0, ring=nc.sync, split=True)
            fc_attn_final(1, 1, ps1, ring=nc.scalar, split=True)
            fc_attn_final(1, 2, ps2, ring=nc.sync, split=True)
            fc_attn_final(1, 3, ps3, ring=nc.scalar, split=True)

    _split_sync_waits(nc, mybir)
    return nc


def _get_nc():
    if "nc" not in _CACHE:
        _CACHE["nc"] = _build()
    return _CACHE["nc"]


def kernel(Q, V, Wq, bq, Wv, bv, Wf, bf, _trace=False):
    from concourse.bass_utils import run_bass_kernel_spmd

    nc = _get_nc()
    Q = np.ascontiguousarray(np.asarray(Q, dtype=np.float32))
    V = np.ascontiguousarray(np.asarray(V, dtype=np.float32))
    shared = {
        "Wq": np.ascontiguousarray(np.asarray(Wq, np.float32)),
        "bq": np.ascontiguousarray(np.asarray(bq, np.float32)),
        "Wv": np.ascontiguousarray(np.asarray(Wv, np.float32)),
        "bv": np.ascontiguousarray(np.asarray(bv, np.float32)),
        "Wf": np.ascontiguousarray(np.asarray(Wf, np.float32)),
        "bf": np.ascontiguousarray(np.asarray(bf, np.float32)),
    }
    in_maps = []
    for c in range(NCORES):
        m = {"Q": Q[c * BPC:(c + 1) * BPC], "V": V[c * BPC:(c + 1) * BPC]}
        m.update(shared)
        in_maps.append(m)

    res = run_bass_kernel_spmd(nc, in_maps, core_ids=list(range(NCORES)),
                               trace=_trace)
    out = np.concatenate([res.results[c]["O"] for c in range(NCORES)], axis=0)
    if _trace:
        _CACHE["last_exec_time_ns"] = res.exec_time_ns
        _CACHE["last_res"] = res
    return out
